# revision 1
# baseline (speedup 1.0000x reference)
"""Trainium2 Bass kernel for nn_Net_74552042324489.

Data-parallel over batch n=8 across 8 NeuronCores (1 sample/core).
Per-core pipeline:
  cam = fc8_w @ _4            -> norm/suppress -> camT5 = [bg|fg supp|ones]^T
  x2r = bilinear(x2,112->56)   (stride-2 DVE trick, align_corners)
  f8_3 = relu(f83_w @ x2r)
  f8_4 = relu(f84_w @ deep3)
  x_s = bilinear(x,448->56)    (dense resize-matrix matmuls on PE)
  f = [f8_4; f8_3; x_s]        (channel-permuted; qk weights permuted to match)
  q,k = Wqk @ f
  Attention: S blocked [h=128p, k free]; exp on ScalarE (no max-sub needed:
  |S|<~30); second matmul lhsT=[camT|ones] fuses numerator + softmax denom;
  divide at the end.  Output [4, 3136] per core.
"""

import os
import sys

sys.path.insert(0, "/opt/trn_rl_repo")

from contextlib import ExitStack

import numpy as np

import concourse.bass as bass
import concourse.tile as tile
from concourse import bacc, mybir
from concourse.bass_utils import run_bass_kernel_spmd
from concourse.masks import make_identity

F32 = mybir.dt.float32
BF16 = mybir.dt.bfloat16
F32R = mybir.dt.float32r
AF = mybir.ActivationFunctionType
ALU = mybir.AluOpType

HW = 3136  # 56*56
N_CORES = 8

_CACHE = {}


def _resize_mat(h_in: int, h_out: int) -> np.ndarray:
    """Dense [h_in, h_out] bilinear align_corners=True resize matrix."""
    ys = np.linspace(0.0, h_in - 1.0, h_out).astype(np.float32)
    y0 = np.floor(ys).astype(np.int64)
    y1 = np.minimum(y0 + 1, h_in - 1)
    w = (ys - y0).astype(np.float32)
    R = np.zeros((h_in, h_out), np.float32)
    for i in range(h_out):
        R[y0[i], i] += 1.0 - w[i]
        R[y1[i], i] += w[i]
    return R


def _resize_coeffs_112() -> tuple[np.ndarray, np.ndarray]:
    """Per-output-col (0..54) weights for the stride-2 112->56 resize."""
    ys = np.linspace(0.0, 111.0, 56).astype(np.float32)
    y0 = np.floor(ys).astype(np.int64)
    w = (ys - y0).astype(np.float32)
    # structural property (verified): y0[i] == 2i for i < 55; y0[55] == 111
    a = (1.0 - w).astype(np.float32)  # weight of in[2i]
    b = w.astype(np.float32)          # weight of in[2i+1]
    return a, b


def _build_program():
    nc = bacc.Bacc(
        "TRN2", target_bir_lowering=False, debug=False, num_devices=N_CORES
    )

    # ---- DRAM I/O ----
    d_x4 = nc.dram_tensor("x4", [512, HW], F32, kind="ExternalInput")
    d_deep3 = nc.dram_tensor("deep3", [320, HW], F32, kind="ExternalInput")
    d_x2 = nc.dram_tensor("x2", [128, 112 * 112], F32, kind="ExternalInput")
    d_x = nc.dram_tensor("x", [3, 448, 448], BF16, kind="ExternalInput")
    d_fc8T = nc.dram_tensor("fc8T", [512, 4], F32, kind="ExternalInput")
    d_f83T = nc.dram_tensor("f83T", [128, 64], F32, kind="ExternalInput")
    d_f84T = nc.dram_tensor("f84T", [320, 128], F32, kind="ExternalInput")
    d_qkA = nc.dram_tensor("qkA", [128, 384], F32, kind="ExternalInput")
    d_qkB = nc.dram_tensor("qkB", [67, 384], F32, kind="ExternalInput")
    d_a112 = nc.dram_tensor("a112", [128, 56], F32, kind="ExternalInput")
    d_b112 = nc.dram_tensor("b112", [128, 56], F32, kind="ExternalInput")
    d_rh = nc.dram_tensor("rh448", [448, 56], BF16, kind="ExternalInput")
    d_rw = nc.dram_tensor("rw448", [448, 56], BF16, kind="ExternalInput")
    d_out = nc.dram_tensor("out", [4, HW], F32, kind="ExternalOutput")

    EPS = 1e-05

    with tile.TileContext(nc) as tc, ExitStack() as top:
        wpool = top.enter_context(tc.tile_pool(name="wpool", bufs=1))
        persist = top.enter_context(tc.tile_pool(name="persist", bufs=1))
        small = top.enter_context(tc.tile_pool(name="small", bufs=2))

        # ---- weights to SBUF (ordered by first use) ----
        a112 = wpool.tile([128, 56], F32, tag="a112")
        nc.sync.dma_start(a112[:], d_a112.ap())
        b112 = wpool.tile([128, 56], F32, tag="b112")
        nc.sync.dma_start(b112[:], d_b112.ap())
        fc8T = wpool.tile([128, 4, 4], F32, tag="fc8T")
        nc.sync.dma_start(fc8T[:], d_fc8T.ap().rearrange("(k p) o -> p k o", p=128))
        f84T_0 = wpool.tile([128, 128], F32, tag="f84T0")
        nc.sync.dma_start(f84T_0[:], d_f84T.ap()[0:128, :])
        f84T_1 = wpool.tile([128, 128], F32, tag="f84T1")
        nc.sync.dma_start(f84T_1[:], d_f84T.ap()[128:256, :])
        f84T_2 = wpool.tile([64, 128], F32, tag="f84T2")
        nc.sync.dma_start(f84T_2[:], d_f84T.ap()[256:320, :])
        f83T = wpool.tile([128, 64], F32, tag="f83T")
        nc.sync.dma_start(f83T[:], d_f83T.ap())
        rh = wpool.tile([112, 4, 56], BF16, tag="rh")
        nc.sync.dma_start(rh[:], d_rh.ap().rearrange("(k p) o -> p k o", p=112))
        rw = wpool.tile([112, 4, 56], BF16, tag="rw")
        nc.sync.dma_start(rw[:], d_rw.ap().rearrange("(k p) o -> p k o", p=112))
        qkA = wpool.tile([128, 384], F32, tag="qkA")
        nc.sync.dma_start(qkA[:], d_qkA.ap())
        qkB = wpool.tile([67, 384], F32, tag="qkB")
        nc.sync.dma_start(qkB[:], d_qkB.ap())
        ident = wpool.tile([128, 128], F32, tag="ident")
        make_identity(nc, ident[:])
        f84R_0 = wpool.tile([128, 128], F32R, tag="f84R0")
        nc.vector.tensor_copy(f84R_0[:], f84T_0[:])
        f84R_1 = wpool.tile([128, 128], F32R, tag="f84R1")
        nc.vector.tensor_copy(f84R_1[:], f84T_1[:])
        f84R_2 = wpool.tile([64, 128], F32R, tag="f84R2")
        nc.vector.tensor_copy(f84R_2[:], f84T_2[:])
        f83R = wpool.tile([128, 64], F32R, tag="f83R")
        nc.vector.tensor_copy(f83R[:], f83T[:])
        qkAR = wpool.tile([128, 384], F32R, tag="qkAR")
        nc.vector.tensor_copy(qkAR[:], qkA[:])
        qkBR = wpool.tile([67, 384], F32R, tag="qkBR")
        nc.vector.tensor_copy(qkBR[:], qkB[:])

        camT5 = persist.tile([128, 125], BF16, tag="camT5")  # 25 h-blocks x 5
        f_a = persist.tile([128, HW], F32R, tag="f_a")  # = f8_4
        f_b = persist.tile([67, HW], F32R, tag="f_b")  # = [f8_3(64); x_s(3)]
        qA = persist.tile([128, HW], BF16, tag="qA")
        qB = persist.tile([64, HW], BF16, tag="qB")
        kA = persist.tile([128, HW], BF16, tag="kA")
        kB = persist.tile([64, HW], BF16, tag="kB")
        out_sb = persist.tile([4, HW], F32, tag="out_sb")

        # h-block partition sizes: 24 x 128 + 1 x 64
        HBLK = [(i * 128, 128) for i in range(24)] + [(3072, 64)]
        # free-dim 512 chunks of 3136: 6 x 512 + 64
        NCH = [(i * 512, 512) for i in range(6)] + [(3072, 64)]

        # ================= P2: x2 -> x2r (stride-2 bilinear) =================
        # Emitted first so the long DVE resize chain overlaps the PE conv
        # phases (cam, f8_4) that only need DMA inputs.
        with tc.tile_pool(name="p2w", bufs=1) as p2w, \
             tc.tile_pool(name="p2s", bufs=2) as p2s, \
             tc.tile_pool(name="p2r", bufs=1) as p2r:
            x2w = p2w.tile([128, 112 * 56], F32, tag="x2w")  # after W-resize
            x2wv = x2w[:].rearrange("p (h w) -> p h w", h=112)
            HC = 14  # h rows per W-stage chunk
            for hc in range(112 // HC):
                st = p2s.tile([128, HC * 112], F32, tag="x2st")
                nc.sync.dma_start(
                    st[:], d_x2.ap()[:, hc * HC * 112:(hc + 1) * HC * 112]
                )
                sv = st[:].rearrange("p (h w) -> p h w", h=HC)
                dst = x2wv[:, hc * HC:(hc + 1) * HC, :]
                even = sv[:, :, 0:110:2]   # 55 taps
                odd = sv[:, :, 1:111:2]
                abc = a112[:, 0:55].unsqueeze(1).broadcast_to([128, HC, 55])
                bbc = b112[:, 0:55].unsqueeze(1).broadcast_to([128, HC, 55])
                t1 = p2s.tile([128, HC, 55], F32, tag="t1")
                nc.vector.tensor_tensor(t1[:], even, abc, op=ALU.mult)
                t2 = p2s.tile([128, HC, 55], F32, tag="t2")
                nc.vector.tensor_tensor(t2[:], odd, bbc, op=ALU.mult)
                nc.vector.tensor_tensor(dst[:, :, 0:55], t1[:], t2[:], op=ALU.add)
                nc.vector.tensor_copy(dst[:, :, 55:56], sv[:, :, 111:112])

            x2r = p2r.tile([128, HW], F32R, tag="x2r")
            x2rv = x2r[:].rearrange("p (h w) -> p h w", h=56)
            for jc, jl in ((0, 28), (28, 27)):
                everow = x2wv[:, 2 * jc:2 * (jc + jl) - 1:2, :]
                oddrow = x2wv[:, 2 * jc + 1:2 * (jc + jl):2, :]
                arow = a112[:, jc:jc + jl].unsqueeze(2).broadcast_to([128, jl, 56])
                brow = b112[:, jc:jc + jl].unsqueeze(2).broadcast_to([128, jl, 56])
                t3 = p2s.tile([128, 28, 56], F32, tag="t1")
                nc.vector.tensor_tensor(t3[:, 0:jl, :], everow, arow, op=ALU.mult)
                t4 = p2s.tile([128, 28, 56], F32, tag="t2")
                nc.vector.tensor_tensor(t4[:, 0:jl, :], oddrow, brow, op=ALU.mult)
                nc.vector.tensor_tensor(
                    x2rv[:, jc:jc + jl, :], t3[:, 0:jl, :], t4[:, 0:jl, :], op=ALU.add
                )
            nc.vector.tensor_copy(x2rv[:, 55:56, :], x2wv[:, 111:112, :])


            # ---- f8_4 = relu(f84R.T @ deep3) -> f_a (PE work under resize) ----
            with tc.tile_pool(name="p5s", bufs=4) as p5s, \
                 tc.tile_pool(name="p5p", bufs=4,
                              space=bass.MemorySpace.PSUM) as p5p:
                DCH = [(0, 128), (128, 128), (256, 64)]
                for no, nl in NCH:
                    fp = p5p.tile([128, 512], F32, tag="f4psum")
                    for ci, (co, cl) in enumerate(DCH):
                        st5 = p5s.tile([128, 512], F32, tag="d3st")
                        if no == 0:
                            for sl in range(cl // 32):
                                nc.sync.dma_start(
                                    st5[32 * sl:32 * (sl + 1), 0:nl],
                                    d_deep3.ap()[co + 32 * sl:co + 32 * (sl + 1),
                                                 no:no + nl],
                                )
                        else:
                            nc.sync.dma_start(
                                st5[0:cl, 0:nl],
                                d_deep3.ap()[co:co + cl, no:no + nl],
                            )
                        d3r = p5s.tile([128, 512], F32R, tag="d3r")
                        nc.scalar.copy(d3r[0:cl, 0:nl], st5[0:cl, 0:nl])
                        w = (f84R_0, f84R_1, f84R_2)[ci]
                        nc.tensor.matmul(
                            fp[:, 0:nl], w[:], d3r[0:cl, 0:nl],
                            start=(ci == 0), stop=(ci == 2),
                        )
                    nc.scalar.activation(f_a[:, no:no + nl], fp[:, 0:nl], AF.Relu)

            # ================= P4: x -> x_s -> f_b[64:67] =================
            with tc.tile_pool(name="p4s", bufs=2) as p4s, \
                 tc.tile_pool(name="p4sb", bufs=1) as p4sb, \
                 tc.tile_pool(name="p4p", bufs=1, space=bass.MemorySpace.PSUM) as p4p:
                xh = p4sb.tile([56, 3, 448], BF16, tag="xh")
                xps = [
                    p4p.tile([56, 448], F32, tag=f"xhp{c}", name=f"xhp{c}")
                    for c in range(3)
                ]
                xdr = d_x.ap().rearrange("c h w -> h c w")
                for hc in range(4):
                    st = p4s.tile([112, 3, 448], BF16, tag="xst")
                    nc.sync.dma_start(st[:], xdr[112 * hc:112 * (hc + 1), :, :])
                    for c in range(3):
                        nc.tensor.matmul(
                            xps[c][:], rh[:, hc, :], st[:, c, :],
                            start=(hc == 0), stop=(hc == 3),
                        )
                for c in range(3):
                    nc.vector.tensor_copy(xh[:, c, :], xps[c][:])

                xhT = p4sb.tile([112, 12, 56], BF16, tag="xhT")
                idb = p4sb.tile([128, 128], BF16, tag="idb")
                nc.vector.tensor_copy(idb[:], ident[:])
                for c in range(3):
                    for wc in range(4):
                        tp = p4p.tile([112, 56], BF16, tag="xtp", bufs=2)
                        nc.tensor.transpose(
                            tp[:], xh[:, c, 112 * wc:112 * (wc + 1)], idb[0:56, 0:56]
                        )
                        nc.vector.tensor_copy(xhT[:, c * 4 + wc, :], tp[:])
                xs3 = p4sb.tile([3, HW], F32, tag="xs3")
                for c in range(3):
                    wp = p4p.tile([56, 56], F32, tag="xwp", bufs=2)
                    for wc in range(4):
                        nc.tensor.matmul(
                            wp[:], xhT[:, c * 4 + wc, :], rw[:, wc, :],
                            start=(wc == 0), stop=(wc == 3),
                        )
                    ws = p4s.tile([56, 56], F32, tag="xws")
                    nc.vector.tensor_copy(ws[:], wp[:])
                    nc.sync.dma_start(xs3[c:c + 1, :], ws[:])
                nc.vector.tensor_copy(f_b[64:67, :], xs3[:])


            # ---- cam = fc8T.T @ _4 (x4 streamed last; cam needed only at P7) ----
            with tc.tile_pool(name="p1s", bufs=4) as p1s, \
                 tc.tile_pool(name="p1p", bufs=2, space=bass.MemorySpace.PSUM) as p1p, \
                 tc.tile_pool(name="p1sb", bufs=1) as p1sb:
                cam = p1sb.tile([4, HW], F32, tag="cam")
                for no, nl in NCH:
                    cp = p1p.tile([4, 512], F32, tag="campsum")
                    for ck in range(4):
                        st = p1s.tile([128, 512], F32, tag="x4st")
                        nc.sync.dma_start(
                            st[:, 0:nl],
                            d_x4.ap()[128 * ck:128 * (ck + 1), no:no + nl],
                        )
                        nc.tensor.matmul(
                            cp[:, 0:nl], fc8T[:, ck, :], st[:, 0:nl],
                            start=(ck == 0), stop=(ck == 3),
                        )
                    nc.scalar.copy(cam[:, no:no + nl], cp[:, 0:nl])

                # ---- P3: f8_3 = relu(f83T.T @ x2r) -> f_b[0:64] ----
                with tc.tile_pool(name="p3p", bufs=2,
                                  space=bass.MemorySpace.PSUM) as p3p:
                    for no, nl in NCH:
                        fp3 = p3p.tile([64, 512], F32, tag="f3psum")
                        nc.tensor.matmul(
                            fp3[:, 0:nl], f83R[:], x2r[:, no:no + nl],
                            start=True, stop=True,
                        )
                        nc.scalar.activation(
                            f_b[0:64, no:no + nl], fp3[:, 0:nl], AF.Relu
                        )

                # ---- P1b: normalize, transpose, fg-suppress -> camT5 ----
                mn = small.tile([4, 1], F32, tag="mn")
                mx = small.tile([4, 1], F32, tag="mx")
                nc.vector.tensor_reduce(
                    mn[:], cam[:], axis=mybir.AxisListType.X, op=ALU.min
                )
                nc.vector.tensor_reduce(
                    mx[:], cam[:], axis=mybir.AxisListType.X, op=ALU.max
                )
                rng = small.tile([4, 1], F32, tag="rng")
                nc.vector.tensor_tensor(rng[:], mx[:], mn[:], op=ALU.subtract)
                nc.vector.tensor_scalar_add(rng[:], rng[:], EPS)
                rs = small.tile([4, 1], F32, tag="rs")
                nc.vector.reciprocal(rs[:], rng[:])
                norm = p1sb.tile([4, HW], F32, tag="norm")
                nc.vector.tensor_scalar(
                    norm[:], cam[:], mn[:], rs[:], op0=ALU.subtract, op1=ALU.mult
                )

                camTall = p1sb.tile([128, 25, 4], F32, tag="camTall")
                nc.vector.memset(camTall[64:128, 24, :], 0.0)
                for bi, (ho, hl) in enumerate(HBLK):
                    tp = p1p.tile([128, 4], F32, tag="tpsum")
                    nc.tensor.transpose(
                        tp[0:hl, :], norm[:, ho:ho + hl], ident[0:4, 0:4]
                    )
                    nc.vector.tensor_copy(camTall[0:hl, bi, :], tp[0:hl, :])
                # vectorized over all 25 blocks at once
                c5v = camT5[:].rearrange("p (b f) -> p b f", f=5)
                nc.vector.memset(c5v[:, :, 4], 1.0)
                fm = p1sb.tile([128, 25], F32, tag="fm")
                nc.vector.tensor_reduce(
                    fm[:], camTall[:, :, 1:4], axis=mybir.AxisListType.X, op=ALU.max
                )
                nc.vector.tensor_scalar(
                    c5v[:, :, 0], fm[:], -1.0, 1.0, op0=ALU.mult, op1=ALU.add
                )
                msk = p1sb.tile([128, 25, 3], F32, tag="msk")
                fmb = fm[:].unsqueeze(2).broadcast_to([128, 25, 3])
                nc.vector.tensor_tensor(
                    msk[:], camTall[:, :, 1:4], fmb, op=ALU.is_ge
                )
                nc.vector.tensor_tensor(
                    c5v[:, :, 1:4], camTall[:, :, 1:4], msk[:], op=ALU.mult
                )

        # ================= P6: q, k =================
        with tc.tile_pool(name="p6p", bufs=4, space=bass.MemorySpace.PSUM) as p6p:
            MCH = [(qA, 0, 128), (qB, 128, 64), (kA, 192, 128), (kB, 320, 64)]
            for dst, mo, ml in MCH:
                for no, nl in NCH:
                    qp = p6p.tile([128, 512], F32, tag="qkpsum")
                    nc.tensor.matmul(
                        qp[0:ml, 0:nl], qkAR[:, mo:mo + ml], f_a[:, no:no + nl],
                        start=True, stop=False,
                    )
                    nc.tensor.matmul(
                        qp[0:ml, 0:nl], qkBR[:, mo:mo + ml], f_b[:, no:no + nl],
                        start=False, stop=True,
                    )
                    nc.vector.tensor_copy(dst[0:ml, no:no + nl], qp[0:ml, 0:nl])

        # ================= P7: attention =================
        with tc.tile_pool(name="p7e", bufs=6) as p7e, \
             tc.tile_pool(name="p7r", bufs=2) as p7r, \
             tc.tile_pool(name="p7s", bufs=2, space=bass.MemorySpace.PSUM) as p7s, \
             tc.tile_pool(name="p7o", bufs=2, space=bass.MemorySpace.PSUM) as p7o:
            # k-superblocks: 3 x 1024 + 1 x 64
            KSUP = [(0, 1024), (1024, 1024), (2048, 1024), (3072, 64)]
            for ko, kl in KSUP:
                nkb = (kl + 511) // 512
                pout = p7o.tile([5, 1024], F32, tag="pout")
                for bi, (ho, hl) in enumerate(HBLK):
                    sp = p7s.tile([128, 1024], F32, tag="spsum")
                    for kb in range(nkb):
                        kbl = min(512, kl - kb * 512)
                        nc.tensor.matmul(
                            sp[0:hl, kb * 512:kb * 512 + kbl], qA[:, ho:ho + hl],
                            kA[:, ko + kb * 512:ko + kb * 512 + kbl],
                            start=True, stop=False,
                        )
                    for kb in range(nkb):
                        kbl = min(512, kl - kb * 512)
                        nc.tensor.matmul(
                            sp[0:hl, kb * 512:kb * 512 + kbl], qB[:, ho:ho + hl],
                            kB[:, ko + kb * 512:ko + kb * 512 + kbl],
                            start=False, stop=True,
                        )
                    et = p7e.tile([128, 1024], BF16, tag="exptile")
                    nc.scalar.activation(et[0:hl, 0:kl], sp[0:hl, 0:kl], AF.Exp)
                    for kb in range(nkb):
                        kbl = min(512, kl - kb * 512)
                        nc.tensor.matmul(
                            pout[:, kb * 512:kb * 512 + kbl],
                            camT5[0:hl, bi * 5:bi * 5 + 5],
                            et[0:hl, kb * 512:kb * 512 + kbl],
                            start=(bi == 0), stop=(bi == 24),
                        )
                ot5 = p7r.tile([5, 1024], F32, tag="ot5")
                nc.vector.tensor_copy(ot5[:, 0:kl], pout[:, 0:kl])
                den = p7r.tile([1, 1024], F32, tag="den")
                nc.sync.dma_start(den[0:1, 0:kl], ot5[4:5, 0:kl])
                rcp = p7r.tile([1, 1024], F32, tag="rcp")
                rsc = p7r.tile([1, 1024], F32, tag="rsc")
                nc.vector.reciprocal_approx_accurate(
                    rcp[0:1, 0:kl], den[0:1, 0:kl], rsc[0:1, 0:kl]
                )
                rb = p7r.tile([4, 1024], F32, tag="rb")
                nc.gpsimd.partition_broadcast(rb[:, 0:kl], rcp[0:1, 0:kl])
                nc.gpsimd.tensor_tensor(
                    out_sb[:, ko:ko + kl], ot5[0:4, 0:kl], rb[:, 0:kl], op=ALU.mult
                )
                nc.sync.dma_start(
                    d_out.ap()[:, ko:ko + kl], out_sb[:, ko:ko + kl]
                )

    nc.compile()
    return nc


def _get_program():
    if "nc" not in _CACHE:
        _CACHE["nc"] = _build_program()
    return _CACHE["nc"]


def _host_prep(inputs: dict) -> list[dict]:
    x = np.ascontiguousarray(np.asarray(inputs["x"], np.float32))
    x2 = np.ascontiguousarray(np.asarray(inputs["x2"], np.float32))
    deep3 = np.ascontiguousarray(np.asarray(inputs["deep3"], np.float32))
    _4 = np.ascontiguousarray(np.asarray(inputs["_4"], np.float32))
    fc8_w = np.asarray(inputs["fc8_w"], np.float32)
    f83_w = np.asarray(inputs["f83_w"], np.float32)
    f84_w = np.asarray(inputs["f84_w"], np.float32)
    f91_w = np.asarray(inputs["f91_w"], np.float32)
    f92_w = np.asarray(inputs["f92_w"], np.float32)

    n = x.shape[0]
    fc8T = np.ascontiguousarray(fc8_w.T)  # [512, 4]
    f83T = np.ascontiguousarray(f83_w.T)  # [128, 64]
    f84T = np.ascontiguousarray(f84_w.T)  # [320, 128]
    # f channel permutation: [f8_4 (128), f8_3 (64), x_s (3)]
    perm = np.concatenate([np.arange(67, 195), np.arange(3, 67), np.arange(3)])
    wqk = np.concatenate([f91_w, f92_w], axis=0)[:, perm]  # [384, 195]
    wqkT = np.ascontiguousarray(wqk.T)  # [195, 384]
    qkA = np.ascontiguousarray(wqkT[0:128])
    qkB = np.ascontiguousarray(wqkT[128:195])
    a112, b112 = _resize_coeffs_112()
    import ml_dtypes

    BFNP = ml_dtypes.bfloat16
    a112 = np.ascontiguousarray(np.broadcast_to(a112, (128, 56)))
    b112 = np.ascontiguousarray(np.broadcast_to(b112, (128, 56)))
    rh448 = _resize_mat(448, 56).astype(BFNP)
    rw448 = rh448  # same matrix for H and W (448x448 -> 56x56)
    x = x.astype(BFNP)

    shared = {
        "fc8T": fc8T, "f83T": f83T, "f84T": f84T, "qkA": qkA, "qkB": qkB,
        "a112": a112, "b112": b112, "rh448": rh448, "rw448": rw448,
    }
    in_maps = []
    for i in range(n):
        m = dict(shared)
        m["x4"] = _4[i].reshape(512, HW)
        m["deep3"] = deep3[i].reshape(320, HW)
        m["x2"] = x2[i].reshape(128, 112 * 112)
        m["x"] = x[i]
        in_maps.append(m)
    return in_maps


def _install_ntff_hook() -> bool:
    """Register the NTFF profile hook that the agent image's antenv lacks."""
    try:
        import types

        import antenv

        if "antenv.axon_hooks" not in sys.modules:
            mod = types.ModuleType("antenv.axon_hooks")
            store = {"h": None}
            mod.set_axon_ntff_profile_hook = lambda h: store.update(h=h)
            mod.get_axon_ntff_profile_hook = lambda: store["h"]
            sys.modules["antenv.axon_hooks"] = mod
            antenv.axon_hooks = mod
            from trn_agent_boot.trn_boot import _ntff_profile_via_ctypes

            hook = _ntff_profile_via_ctypes("/opt/axon/libaxon_pjrt.so")
            if hook is None:
                return False
            mod.set_axon_ntff_profile_hook(hook)
        return sys.modules["antenv.axon_hooks"].get_axon_ntff_profile_hook() is not None
    except Exception as e:  # profiling is best-effort
        print(f"ntff hook install failed: {e}", file=sys.stderr)
        return False


def kernel(**inputs) -> np.ndarray:
    nc = _get_program()
    in_maps = _host_prep(inputs)
    trace = bool(int(os.environ.get("KERNEL_PROFILE", "0")))
    if trace:
        trace = _install_ntff_hook()
    res = run_bass_kernel_spmd(nc, in_maps, core_ids=list(range(N_CORES)),
                               trace=trace)
    _CACHE["last_result"] = res
    out = np.stack([r["out"] for r in res.results]).reshape(8, 4, 56, 56)
    return out.astype(np.float32)



# revision 43
# speedup vs baseline: 1.4142x; 1.4142x over previous
"""Trainium2 Bass kernel for nn_Net_74552042324489.

Data-parallel over batch n=8 across 8 NeuronCores (1 sample/core).
v4 highlights (baseline 345us -> v3 233us -> this):
  - All small weights ship in 2 packed DMAs (the v3 11-DMA weight chain
    serialized the scalar queue for 16us and stalled everything).
  - x2/deep3/f84/f83/resize-coeffs in bf16; x2 resize arithmetic in bf16
    (DVE 2x H-stage).  W-stage: DVE does t1-mult+add, GpSimd does t2-mult
    per chunk; H runs as 4 quarter-pieces pipelined behind W.
  - cam needs exact-fp32 fidelity (the fg-suppression argmax flips under
    any rounding — even f32r/tf32 — and each flip is a full-magnitude
    error): _4 ships fp32 and cam runs as fp32 matmuls like the baseline.
  - f8_3 and the padded-M q/k A-passes pipeline chunk-wise behind the
    resize quarters; B-passes follow when x_s and the relus land.
  - Attention: 4 x 784-col superblocks; all S matmuls K=128 (zero-padded
    qB/kB); exp -> bf16 et ring; the camT5 matmuls run as batch-5 groups
    interleaved into the S stream with a 9-block lag (plus a 5-block
    carry into the next superblock) so ScalarE's exp throughput and the
    PE stream overlap fully.  camT5 transposes are deferred into the
    first superblock's exp-gated bubbles.  Epilogue: DMA-broadcast of
    the fused denominator, DVE reciprocal+multiply, DMA out.
"""

import os
import sys

sys.path.insert(0, "/opt/trn_rl_repo")

from contextlib import ExitStack

import numpy as np

import concourse.bass as bass
import concourse.tile as tile
from concourse import bacc, mybir
from concourse.bass_utils import run_bass_kernel_spmd
from concourse.masks import make_identity

F32 = mybir.dt.float32
BF16 = mybir.dt.bfloat16
F32R = mybir.dt.float32r
AF = mybir.ActivationFunctionType
ALU = mybir.AluOpType

HW = 3136  # 56*56
N_CORES = 8

_CACHE = {}


def _resize_mat(h_in: int, h_out: int) -> np.ndarray:
    """Dense [h_in, h_out] bilinear align_corners=True resize matrix."""
    ys = np.linspace(0.0, h_in - 1.0, h_out).astype(np.float32)
    y0 = np.floor(ys).astype(np.int64)
    y1 = np.minimum(y0 + 1, h_in - 1)
    w = (ys - y0).astype(np.float32)
    R = np.zeros((h_in, h_out), np.float32)
    for i in range(h_out):
        R[y0[i], i] += 1.0 - w[i]
        R[y1[i], i] += w[i]
    return R


def _resize_coeffs_112() -> tuple[np.ndarray, np.ndarray]:
    """Per-output-col (0..54) weights for the stride-2 112->56 resize."""
    ys = np.linspace(0.0, 111.0, 56).astype(np.float32)
    y0 = np.floor(ys).astype(np.int64)
    w = (ys - y0).astype(np.float32)
    # structural property (verified): y0[i] == 2i for i < 55; y0[55] == 111
    a = (1.0 - w).astype(np.float32)  # weight of in[2i]
    b = w.astype(np.float32)          # weight of in[2i+1]
    return a, b


def _build_program():
    nc = bacc.Bacc(
        "TRN2", target_bir_lowering=False, debug=False, num_devices=N_CORES
    )

    # ---- DRAM I/O ----
    d_x4 = nc.dram_tensor("x4", [512, HW], F32, kind="ExternalInput")
    d_deep3 = nc.dram_tensor("deep3", [320, HW], BF16, kind="ExternalInput")
    d_x2 = nc.dram_tensor("x2", [128, 112 * 112], BF16, kind="ExternalInput")
    d_x = nc.dram_tensor("x", [3, 448, 448], BF16, kind="ExternalInput")
    d_wf32 = nc.dram_tensor("wf32", [128, 912], F32, kind="ExternalInput")
    d_wbf = nc.dram_tensor("wbf", [128, 1008], BF16, kind="ExternalInput")
    d_out = nc.dram_tensor("out", [4, HW], F32, kind="ExternalOutput")

    EPS = 1e-05

    with tile.TileContext(nc) as tc, ExitStack() as top:
        wpool = top.enter_context(tc.tile_pool(name="wpool", bufs=1))
        persist = top.enter_context(tc.tile_pool(name="persist", bufs=1))
        small = top.enter_context(tc.tile_pool(name="small", bufs=2))
        p1stack = ExitStack()
        p1sb = p1stack.enter_context(tc.tile_pool(name="p1sb", bufs=1))
        x4stack = ExitStack()
        x4pool = x4stack.enter_context(tc.tile_pool(name="x4pool", bufs=1))

        # ================= x2 chunk DMAs (sync queue, t=0) =============
        p2stack = ExitStack()
        p2s = p2stack.enter_context(tc.tile_pool(name="p2s", bufs=2))
        p2w = p2stack.enter_context(tc.tile_pool(name="p2w", bufs=1))
        HC = 14  # h rows per W-stage chunk
        x2st = []
        for hc in range(112 // HC):
            st = p2s.tile([128, HC * 112], BF16, tag="x2st", bufs=3,
                          name=f"x2st{hc}")
            nc.sync.dma_start(
                st[:], d_x2.ap()[:, hc * HC * 112:(hc + 1) * HC * 112]
            )
            x2st.append(st)

        # ---- packed weights: 2 DMAs on the scalar HWDGE queue ----
        wf = wpool.tile([128, 912], F32, tag="wf")
        nc.scalar.dma_start(wf[:], d_wf32.ap())
        wb = wpool.tile([128, 1008], BF16, tag="wb")
        nc.scalar.dma_start(wb[:], d_wbf.ap())
        f84B = (wb[:, 0:128], wb[:, 128:256], wb[0:64, 256:384])
        rh = wb[0:112, 384:608].rearrange("p (k o) -> p k o", k=4)
        rw = wb[0:112, 608:832].rearrange("p (k o) -> p k o", k=4)
        a112b = wb[:, 832:888]
        b112b = wb[:, 888:944]
        f83b = wb[:, 944:1008]
        qkAS = wf[:, 0:448]
        qkBS = wf[0:67, 448:896]
        fc8S = wf[:, 896:912]
        ident = wpool.tile([128, 128], F32, tag="ident")
        make_identity(nc, ident[:])

        # f32r weights: engine casts (DVE stream head; inputs land ~1.5us)
        qkAR = wpool.tile([128, 448], F32R, tag="qkAR")
        nc.vector.tensor_copy(qkAR[:], qkAS)
        qkBR = wpool.tile([67, 448], F32R, tag="qkBR")
        nc.vector.tensor_copy(qkBR[:], qkBS)
        fc8V = fc8S.rearrange("p (k o) -> p k o", k=4)  # [128, 4, 4] fp32

        # ---- early persistent activations ----
        camT5 = persist.tile([128, 125], BF16, tag="camT5")  # 25 blk x 5
        f_a = persist.tile([128, HW], F32R, tag="f_a")  # = f8_4
        f_b = persist.tile([67, HW], F32R, tag="f_b")  # = [f8_3(64); x_s(3)]
        x2r = persist.tile([128, HW], BF16, tag="x2r")
        x2rv = x2r[:].rearrange("p (h w) -> p h w", h=56)

        # h-block partition sizes: 24 x 128 + 1 x 64
        HBLK = [(i * 128, 128) for i in range(24)] + [(3072, 64)]
        # free-dim 512 chunks of 3136: 6 x 512 + 64
        NCH = [(i * 512, 512) for i in range(6)] + [(3072, 64)]

        # ================= x2 resize emit helpers =================
        x2w = p2w.tile([128, 112 * 56], BF16, tag="x2w")  # after W-resize
        x2wv = x2w[:].rearrange("p (h w) -> p h w", h=112)

        def emit_wchunk(hc):
            # t1-mult + add on DVE; t2-mult on GpSimd (op-level split)
            sv = x2st[hc][:].rearrange("p (h w) -> p h w", h=HC)
            dst = x2wv[:, hc * HC:(hc + 1) * HC, :]
            even = sv[:, :, 0:110:2]   # 55 taps
            odd = sv[:, :, 1:111:2]
            abc = a112b[:, 0:55].unsqueeze(1).broadcast_to([128, HC, 55])
            bbc = b112b[:, 0:55].unsqueeze(1).broadcast_to([128, HC, 55])
            t1 = p2s.tile([128, HC, 55], BF16, tag="t1", name=f"t1_{hc}")
            nc.vector.tensor_tensor(t1[:], even, abc, op=ALU.mult)
            t2 = p2s.tile([128, HC, 55], BF16, tag="t2", name=f"t2_{hc}")
            nc.gpsimd.tensor_tensor(t2[:], odd, bbc, op=ALU.mult)
            nc.vector.tensor_tensor(dst[:, :, 0:55], t1[:], t2[:], op=ALU.add)
            nc.vector.tensor_copy(dst[:, :, 55:56], sv[:, :, 111:112])

        def emit_hquarter(q):
            j0, jl = 14 * q, (14 if q < 3 else 13)
            everow = x2wv[:, 2 * j0:2 * (j0 + jl) - 1:2, :]
            oddrow = x2wv[:, 2 * j0 + 1:2 * (j0 + jl):2, :]
            arow = a112b[:, j0:j0 + jl].unsqueeze(2).broadcast_to([128, jl, 56])
            brow = b112b[:, j0:j0 + jl].unsqueeze(2).broadcast_to([128, jl, 56])
            t3 = p2s.tile([128, 14, 56], BF16, tag="th1", name=f"th1_{q}")
            nc.vector.tensor_tensor(t3[:, 0:jl, :], everow, arow, op=ALU.mult)
            t4 = p2s.tile([128, 14, 56], BF16, tag="th2", name=f"th2_{q}")
            nc.vector.tensor_tensor(t4[:, 0:jl, :], oddrow, brow, op=ALU.mult)
            nc.vector.tensor_tensor(
                x2rv[:, j0:j0 + jl, :], t3[:, 0:jl, :], t4[:, 0:jl, :],
                op=ALU.add
            )
            if q == 3:
                nc.vector.tensor_copy(x2rv[:, 55:56, :], x2wv[:, 111:112, :])

        for hc in range(4):
            emit_wchunk(hc)

        # ======== f8_4 = relu(f84B.T @ deep3): slab-major, scalar q ========
        with tc.tile_pool(name="p5s", bufs=1) as p5s, \
             tc.tile_pool(name="p5p", bufs=1,
                          space=bass.MemorySpace.PSUM) as p5p:
            DCH = [(0, 128), (128, 128), (256, 64)]
            slabs = []
            for ci, (co, cl) in enumerate(DCH):
                sl = p5s.tile([cl, HW], BF16, tag=f"d3s{ci}", name=f"d3s{ci}")
                nc.scalar.dma_start(sl[:], d_deep3.ap()[co:co + cl, :])
                slabs.append(sl)
            fps = [
                p5p.tile([128, 512], F32, tag=f"f4psum{i}", name=f"f4psum{i}")
                for i in range(len(NCH))
            ]
            for ci, (co, cl) in enumerate(DCH):
                for (no, nl), fp in zip(NCH, fps):
                    nc.tensor.matmul(
                        fp[:, 0:nl], f84B[ci], slabs[ci][:, no:no + nl],
                        start=(ci == 0), stop=(ci == 2),
                    )
            for (no, nl), fp in zip(NCH, fps):
                nc.scalar.activation(f_a[:, no:no + nl], fp[:, 0:nl], AF.Relu)

        # ======== x4 slabs (fp32, queues split: s0 scalar, s1/s2 sync,
        # s3 scalar-after-xst) ========
        x4f = []
        for ck in range(4):
            sl = x4pool.tile([128, HW], F32, tag=f"x4f{ck}", name=f"x4f{ck}")
            x4f.append(sl)
        for ck, eng in ((0, nc.scalar), (1, nc.sync), (2, nc.sync)):
            eng.dma_start(x4f[ck][:], d_x4.ap()[128 * ck:128 * (ck + 1), :])

        # ================= P4: x -> x_s -> f_b[64:67] =================
        # copies routed to ScalarE so the DVE stream stays on the resize
        with tc.tile_pool(name="p4s", bufs=2) as p4s, \
             tc.tile_pool(name="p4sb", bufs=1) as p4sb, \
             tc.tile_pool(name="p4p", bufs=1, space=bass.MemorySpace.PSUM) as p4p:
            xh = p4sb.tile([56, 3, 448], BF16, tag="xh")
            xps = [
                p4p.tile([56, 448], F32, tag=f"xhp{c}", name=f"xhp{c}")
                for c in range(3)
            ]
            xdr = d_x.ap().rearrange("c h w -> h c w")
            for hc in range(4):
                st = p4s.tile([112, 3, 448], BF16, tag="xst")
                nc.scalar.dma_start(st[:], xdr[112 * hc:112 * (hc + 1), :, :])
                for c in range(3):
                    nc.tensor.matmul(
                        xps[c][:], rh[:, hc, :], st[:, c, :],
                        start=(hc == 0), stop=(hc == 3),
                    )
            for c in range(3):
                nc.scalar.copy(xh[:, c, :], xps[c][:])

            xhT = p4sb.tile([112, 12, 56], BF16, tag="xhT")
            idb = p4sb.tile([128, 128], BF16, tag="idb")
            nc.scalar.copy(idb[:], ident[:])
            for c in range(3):
                for wc in range(4):
                    tp = p4p.tile([112, 56], BF16, tag="xtp", bufs=2)
                    nc.tensor.transpose(
                        tp[:], xh[:, c, 112 * wc:112 * (wc + 1)], idb[0:56, 0:56]
                    )
                    nc.scalar.copy(xhT[:, c * 4 + wc, :], tp[:])
            xs3 = p4sb.tile([3, HW], F32, tag="xs3")
            for c in range(3):
                wp = p4p.tile([56, 56], F32, tag="xwp", bufs=2)
                for wc in range(4):
                    nc.tensor.matmul(
                        wp[:], xhT[:, c * 4 + wc, :], rw[:, wc, :],
                        start=(wc == 0), stop=(wc == 3),
                    )
                ws = p4s.tile([56, 56], F32, tag="xws")
                nc.scalar.copy(ws[:], wp[:])
                nc.sync.dma_start(xs3[c:c + 1, :], ws[:])
            nc.scalar.copy(f_b[64:67, :], xs3[:])

        # last x4 slab
        nc.scalar.dma_start(x4f[3][:], d_x4.ap()[384:512, :])

        # late persistent activations (emitted here so their SBUF doesn't
        # collide with the staging/resize pools' peak)
        qA = persist.tile([128, HW], BF16, tag="qA")
        qBp = persist.tile([128, HW], BF16, tag="qBp")  # rows 64:128 zero
        kA = persist.tile([128, HW], BF16, tag="kA")
        kBp = persist.tile([128, HW], BF16, tag="kBp")  # rows 64:128 zero
        nc.gpsimd.memset(qBp[64:128, :], 0.0)
        nc.gpsimd.memset(kBp[64:128, :], 0.0)

        # ---- rest of the resize: W chunks 4-7 + H quarters pipelined ----
        emit_hquarter(0)
        emit_wchunk(4)
        emit_wchunk(5)
        emit_hquarter(1)
        emit_wchunk(6)
        emit_wchunk(7)
        emit_hquarter(2)
        emit_hquarter(3)
        p2stack.close()

        # ======== cam = fc8.T @ _4 (exact fp32, slab-major) ========
        # Slab-major so the PE chews each slab as its DMA lands; fp32
        # moving operand runs at 1/4 rate but exactness is mandatory.
        cam = p1sb.tile([4, HW], F32, tag="cam")
        with tc.tile_pool(name="p1p", bufs=1,
                          space=bass.MemorySpace.PSUM) as p1p:
            cps = [
                p1p.tile([4, 512], F32, tag=f"campsum{i}", name=f"campsum{i}")
                for i in range(len(NCH))
            ]
            for ck in range(4):
                for (no, nl), cp in zip(NCH, cps):
                    nc.tensor.matmul(
                        cp[:, 0:nl], fc8V[:, ck, :], x4f[ck][:, no:no + nl],
                        start=(ck == 0), stop=(ck == 3),
                    )
            for (no, nl), cp in zip(NCH, cps):
                nc.scalar.copy(cam[:, no:no + nl], cp[:, 0:nl])
        x4stack.close()

        # ==== f8_3 (bf16) then q/k (per-dst batched A/B passes) ====
        # All A stationaries are M=128 slices of the 448-wide padded pack,
        # so the only weight-shape transitions are A<->B per dst.
        MCH = [(0, 128), (128, 64), (192, 128), (320, 64)]  # qA qB kA kB
        DSTS = (qA, qBp, kA, kBp)
        with tc.tile_pool(name="p3p", bufs=2,
                          space=bass.MemorySpace.PSUM) as p3p:
            for ci, (no, nl) in enumerate(NCH):
                fp3 = p3p.tile([64, 512], F32, tag="f3psum")
                nc.tensor.matmul(
                    fp3[:, 0:nl], f83b[:], x2r[:, no:no + nl],
                    start=True, stop=True,
                )
                nc.scalar.activation(
                    f_b[0:64, no:no + nl], fp3[:, 0:nl], AF.Relu
                )
        with tc.tile_pool(name="p6p", bufs=1,
                          space=bass.MemorySpace.PSUM) as p6p:
            for di, (mo, ml) in enumerate(MCH):
                qps = []
                for ci, (no, nl) in enumerate(NCH):
                    qp = p6p.tile([128, 512], F32, tag=f"qkp{ci}",
                                  name=f"qkp{di}_{ci}")
                    qps.append(qp)
                    nc.tensor.matmul(
                        qp[:, 0:nl], qkAR[:, mo:mo + 128],
                        f_a[:, no:no + nl],
                        start=True, stop=False,
                    )
                for (no, nl), qp in zip(NCH, qps):
                    nc.tensor.matmul(
                        qp[:, 0:nl], qkBR[:, mo:mo + 128],
                        f_b[:, no:no + nl],
                        start=False, stop=True,
                    )
                for ci, ((no, nl), qp) in enumerate(zip(NCH, qps)):
                    if ci % 2 == 0:
                        nc.vector.tensor_copy(
                            DSTS[di][0:ml, no:no + nl], qp[0:ml, 0:nl]
                        )
                    else:
                        nc.scalar.copy(
                            DSTS[di][0:ml, no:no + nl], qp[0:ml, 0:nl]
                        )

        mn = small.tile([4, 1], F32, tag="mn")
        mx = small.tile([4, 1], F32, tag="mx")
        nc.vector.tensor_reduce(
            mn[:], cam[:], axis=mybir.AxisListType.X, op=ALU.min
        )
        nc.vector.tensor_reduce(
            mx[:], cam[:], axis=mybir.AxisListType.X, op=ALU.max
        )
        rng = small.tile([4, 1], F32, tag="rng")
        nc.vector.tensor_tensor(rng[:], mx[:], mn[:], op=ALU.subtract)
        nc.vector.tensor_scalar_add(rng[:], rng[:], EPS)
        rs = small.tile([4, 1], F32, tag="rs")
        nc.vector.reciprocal(rs[:], rng[:])
        nbias = small.tile([4, 1], F32, tag="nbias")
        nc.vector.tensor_tensor(nbias[:], mn[:], rs[:], op=ALU.mult)
        nc.vector.tensor_scalar(nbias[:], nbias[:], -1.0, 0.0,
                                op0=ALU.mult, op1=ALU.add)
        norm = p1sb.tile([4, HW], F32, tag="norm")
        # norm = cam*rs - mn*rs in one ScalarE op (free affine)
        nc.scalar.activation(norm[:], cam[:], AF.Identity,
                             bias=nbias[:], scale=rs[:])

        camTall = p1sb.tile([128, 25, 4], F32, tag="camTall")
        nc.vector.memset(camTall[64:128, 24, :], 0.0)
        with tc.tile_pool(name="p1t", bufs=2,
                          space=bass.MemorySpace.PSUM) as p1t:
            for bi, (ho, hl) in enumerate(HBLK):
                tp = p1t.tile([128, 4], F32, tag="tpsum")
                nc.tensor.transpose(
                    tp[0:hl, :], norm[:, ho:ho + hl], ident[0:4, 0:4]
                )
                nc.vector.tensor_copy(camTall[0:hl, bi, :], tp[0:hl, :])
        c5v = camT5[:].rearrange("p (b f) -> p b f", f=5)
        nc.vector.memset(c5v[:, :, 4], 1.0)
        fm = p1sb.tile([128, 25], F32, tag="fm")
        nc.vector.tensor_reduce(
            fm[:], camTall[:, :, 1:4],
            axis=mybir.AxisListType.X, op=ALU.max
        )
        nc.vector.tensor_scalar(
            c5v[:, :, 0], fm[:], -1.0, 1.0,
            op0=ALU.mult, op1=ALU.add
        )
        msk = p1sb.tile([128, 25, 3], F32, tag="msk")
        fmb = fm[:].unsqueeze(2).broadcast_to([128, 25, 3])
        nc.vector.tensor_tensor(
            msk[:], camTall[:, :, 1:4], fmb, op=ALU.is_ge
        )
        nc.vector.tensor_tensor(
            c5v[:, :, 1:4], camTall[:, :, 1:4], msk[:], op=ALU.mult
        )

        # ================= P7: attention =================
        with tc.tile_pool(name="p7e", bufs=20) as p7e, \
             tc.tile_pool(name="p7r", bufs=1) as p7r, \
             tc.tile_pool(name="p7s", bufs=3, space=bass.MemorySpace.PSUM) as p7s, \
             tc.tile_pool(name="p7o", bufs=1, space=bass.MemorySpace.PSUM) as p7o:
            KSUP = [(784 * i, 784) for i in range(4)]
            KCH = [(0, 512), (512, 272)]

            def emit_2nd_batch(st, blocks):
                if st["pout"] is None:
                    st["pout"] = p7o.tile([5, 784], F32, tag="pout",
                                          name="pout")
                for pb in blocks:
                    pho, phl = HBLK[pb]
                    for kb, kbl in KCH:
                        nc.tensor.matmul(
                            st["pout"][:, kb:kb + kbl],
                            camT5[0:phl, pb * 5:pb * 5 + 5],
                            st["ets"][pb][0:phl, kb:kb + kbl],
                            start=(pb == 0), stop=(pb == 24),
                        )

            def emit_epilogue(st):
                # broadcast the fused denominator (row 4) to partitions
                # 0-3 via 4 sync-queue SBUF DMAs, reciprocal, multiply
                ko, kl, pout = st["ko"], st["kl"], st["pout"]
                ot5 = p7r.tile([5, 784], F32, tag="ot5")
                nc.vector.tensor_copy(ot5[:], pout[:])
                den4 = p7r.tile([4, 784], F32, tag="den4")
                for r in range(4):
                    nc.sync.dma_start(den4[r:r + 1, :], ot5[4:5, :])
                rcp4 = p7r.tile([4, 784], F32, tag="rcp4")
                rsc4 = p7r.tile([4, 784], F32, tag="rsc4")
                nc.vector.reciprocal_approx_accurate(
                    rcp4[:], den4[:], rsc4[:]
                )
                ob4 = p7r.tile([4, 784], F32, tag="ob4")
                nc.vector.tensor_tensor(
                    ob4[:], ot5[0:4, :], rcp4[:], op=ALU.mult
                )
                nc.sync.dma_start(d_out.ap()[:, ko:ko + kl], ob4[:])

            prev = None  # previous superblock's state
            for si, (ko, kl) in enumerate(KSUP):
                st = {"ko": ko, "kl": kl, "ets": [], "pout": None}
                for bi, (ho, hl) in enumerate(HBLK):
                    sp = p7s.tile([128, 784], F32, tag="spsum")
                    for kb, kbl in KCH:
                        nc.tensor.matmul(
                            sp[0:hl, kb:kb + kbl], qA[:, ho:ho + hl],
                            kA[:, ko + kb:ko + kb + kbl],
                            start=True, stop=False,
                        )
                    for kb, kbl in KCH:
                        nc.tensor.matmul(
                            sp[0:hl, kb:kb + kbl], qBp[:, ho:ho + hl],
                            kBp[:, ko + kb:ko + kb + kbl],
                            start=False, stop=True,
                        )
                    et = p7e.tile([128, 784], BF16, tag="exptile")
                    nc.scalar.activation(et[0:hl, :], sp[0:hl, :], AF.Exp)
                    st["ets"].append(et)
                    if bi == 4 and prev is not None:
                        emit_2nd_batch(prev, range(20, 25))
                        emit_epilogue(prev)
                        prev = None
                    if bi in (9, 14, 19, 24):
                        emit_2nd_batch(st, range(bi - 9, bi - 4))
                prev = st
            emit_2nd_batch(prev, range(20, 25))
            emit_epilogue(prev)
        p1stack.close()

    nc.compile()
    return nc


def _get_program():
    if "nc" not in _CACHE:
        _CACHE["nc"] = _build_program()
    return _CACHE["nc"]


def _host_prep(inputs: dict) -> list[dict]:
    import ml_dtypes

    BFNP = ml_dtypes.bfloat16
    x = np.ascontiguousarray(np.asarray(inputs["x"], np.float32))
    x2 = np.ascontiguousarray(np.asarray(inputs["x2"], np.float32))
    deep3 = np.ascontiguousarray(np.asarray(inputs["deep3"], np.float32))
    _4 = np.ascontiguousarray(np.asarray(inputs["_4"], np.float32))
    fc8_w = np.asarray(inputs["fc8_w"], np.float32)
    f83_w = np.asarray(inputs["f83_w"], np.float32)
    f84_w = np.asarray(inputs["f84_w"], np.float32)
    f91_w = np.asarray(inputs["f91_w"], np.float32)
    f92_w = np.asarray(inputs["f92_w"], np.float32)

    n = x.shape[0]
    # f channel permutation: [f8_4 (128), f8_3 (64), x_s (3)]
    perm = np.concatenate([np.arange(67, 195), np.arange(3, 67), np.arange(3)])
    wqk = np.concatenate([f91_w, f92_w], axis=0)[:, perm]  # [384, 195]
    wqkT = np.ascontiguousarray(wqk.T)  # [195, 384]
    a112, b112 = _resize_coeffs_112()

    # fp32 pack [128, 912]: qkA-pad448 | qkB-pad448 | fc8-pack16
    wf32 = np.zeros((128, 912), np.float32)
    wf32[:, 0:384] = wqkT[0:128]
    wf32[0:67, 448:832] = wqkT[128:195]
    wf32[:, 896:912] = fc8_w.T.reshape(4, 128, 4).transpose(1, 0, 2).reshape(128, 16)

    # bf16 pack [128, 1008]: f84(3x128) | rh(224) | rw(224) | a112 | b112 | f83
    rh448 = _resize_mat(448, 56).astype(np.float32)
    rhp = rh448.reshape(4, 112, 56).transpose(1, 0, 2).reshape(112, 224)
    wbf = np.zeros((128, 1008), np.float32)
    f84T = f84_w.T  # [320, 128]
    wbf[:, 0:128] = f84T[0:128]
    wbf[:, 128:256] = f84T[128:256]
    wbf[0:64, 256:384] = f84T[256:320]
    wbf[0:112, 384:608] = rhp
    wbf[0:112, 608:832] = rhp
    wbf[:, 832:888] = np.broadcast_to(a112, (128, 56))
    wbf[:, 888:944] = np.broadcast_to(b112, (128, 56))
    wbf[:, 944:1008] = f83_w.T  # [128, 64]
    wbf = wbf.astype(BFNP)

    x = x.astype(BFNP)
    x2 = x2.reshape(n, 128, 112 * 112).astype(BFNP)
    deep3 = deep3.reshape(n, 320, HW).astype(BFNP)
    _4 = _4.reshape(n, 512, HW)

    shared = {"wf32": wf32, "wbf": wbf}
    in_maps = []
    for i in range(n):
        m = dict(shared)
        m["x4"] = _4[i]
        m["deep3"] = deep3[i]
        m["x2"] = x2[i]
        m["x"] = x[i]
        in_maps.append(m)
    return in_maps


def _install_ntff_hook() -> bool:
    """Register the NTFF profile hook that the agent image's antenv lacks."""
    try:
        import types

        import antenv

        if "antenv.axon_hooks" not in sys.modules:
            mod = types.ModuleType("antenv.axon_hooks")
            store = {"h": None}
            mod.set_axon_ntff_profile_hook = lambda h: store.update(h=h)
            mod.get_axon_ntff_profile_hook = lambda: store["h"]
            sys.modules["antenv.axon_hooks"] = mod
            antenv.axon_hooks = mod
            from trn_agent_boot.trn_boot import _ntff_profile_via_ctypes

            hook = _ntff_profile_via_ctypes("/opt/axon/libaxon_pjrt.so")
            if hook is None:
                return False
            mod.set_axon_ntff_profile_hook(hook)
        return sys.modules["antenv.axon_hooks"].get_axon_ntff_profile_hook() is not None
    except Exception as e:  # profiling is best-effort
        print(f"ntff hook install failed: {e}", file=sys.stderr)
        return False


def kernel(**inputs) -> np.ndarray:
    nc = _get_program()
    in_maps = _host_prep(inputs)
    trace = bool(int(os.environ.get("KERNEL_PROFILE", "0")))
    if trace:
        trace = _install_ntff_hook()
    res = run_bass_kernel_spmd(nc, in_maps, core_ids=list(range(N_CORES)),
                               trace=trace)
    _CACHE["last_result"] = res
    out = np.stack([r["out"] for r in res.results]).reshape(8, 4, 56, 56)
    return out.astype(np.float32)


# revision 45
# speedup vs baseline: 1.4309x; 1.0118x over previous
"""Trainium2 Bass kernel for nn_Net_74552042324489.

Data-parallel over batch n=8 across 8 NeuronCores (1 sample/core).
v4 highlights (baseline 345us -> v3 233us -> this):
  - All small weights ship in 2 packed DMAs (the v3 11-DMA weight chain
    serialized the scalar queue for 16us and stalled everything).
  - x2/deep3/f84/f83/resize-coeffs in bf16; x2 resize arithmetic in bf16
    (DVE 2x H-stage).  W-stage: DVE does t1-mult+add, GpSimd does t2-mult
    per chunk; H runs as 4 quarter-pieces pipelined behind W.
  - cam needs exact-fp32 fidelity (the fg-suppression argmax flips under
    any rounding — even f32r/tf32 — and each flip is a full-magnitude
    error): _4 ships fp32 and cam runs as fp32 matmuls like the baseline.
  - f8_3 and the padded-M q/k A-passes pipeline chunk-wise behind the
    resize quarters; B-passes follow when x_s and the relus land.
  - Attention: 4 x 784-col superblocks; all S matmuls K=128 (zero-padded
    qB/kB); exp -> bf16 et ring; the camT5 matmuls run as batch-5 groups
    interleaved into the S stream with a 9-block lag (plus a 5-block
    carry into the next superblock) so ScalarE's exp throughput and the
    PE stream overlap fully.  camT5 transposes are deferred into the
    first superblock's exp-gated bubbles.  Epilogue: DMA-broadcast of
    the fused denominator, DVE reciprocal+multiply, DMA out.
"""

import os
import sys

sys.path.insert(0, "/opt/trn_rl_repo")

from contextlib import ExitStack

import numpy as np

import concourse.bass as bass
import concourse.tile as tile
from concourse import bacc, mybir
from concourse.bass_utils import run_bass_kernel_spmd
from concourse.masks import make_identity

F32 = mybir.dt.float32
BF16 = mybir.dt.bfloat16
F32R = mybir.dt.float32r
AF = mybir.ActivationFunctionType
ALU = mybir.AluOpType

HW = 3136  # 56*56
N_CORES = 8

_CACHE = {}


def _resize_mat(h_in: int, h_out: int) -> np.ndarray:
    """Dense [h_in, h_out] bilinear align_corners=True resize matrix."""
    ys = np.linspace(0.0, h_in - 1.0, h_out).astype(np.float32)
    y0 = np.floor(ys).astype(np.int64)
    y1 = np.minimum(y0 + 1, h_in - 1)
    w = (ys - y0).astype(np.float32)
    R = np.zeros((h_in, h_out), np.float32)
    for i in range(h_out):
        R[y0[i], i] += 1.0 - w[i]
        R[y1[i], i] += w[i]
    return R


def _resize_coeffs_112() -> tuple[np.ndarray, np.ndarray]:
    """Per-output-col (0..54) weights for the stride-2 112->56 resize."""
    ys = np.linspace(0.0, 111.0, 56).astype(np.float32)
    y0 = np.floor(ys).astype(np.int64)
    w = (ys - y0).astype(np.float32)
    # structural property (verified): y0[i] == 2i for i < 55; y0[55] == 111
    a = (1.0 - w).astype(np.float32)  # weight of in[2i]
    b = w.astype(np.float32)          # weight of in[2i+1]
    return a, b


def _build_program():
    nc = bacc.Bacc(
        "TRN2", target_bir_lowering=False, debug=False, num_devices=N_CORES
    )

    # ---- DRAM I/O ----
    d_x4 = nc.dram_tensor("x4", [512, HW], F32, kind="ExternalInput")
    d_deep3 = nc.dram_tensor("deep3", [320, HW], BF16, kind="ExternalInput")
    d_x2 = nc.dram_tensor("x2", [128, 112 * 112], BF16, kind="ExternalInput")
    d_x = nc.dram_tensor("x", [3, 448, 448], BF16, kind="ExternalInput")
    d_wf32 = nc.dram_tensor("wf32", [128, 912], F32, kind="ExternalInput")
    d_wbf = nc.dram_tensor("wbf", [128, 1008], BF16, kind="ExternalInput")
    d_out = nc.dram_tensor("out", [4, HW], F32, kind="ExternalOutput")

    EPS = 1e-05

    with tile.TileContext(nc) as tc, ExitStack() as top:
        wpool = top.enter_context(tc.tile_pool(name="wpool", bufs=1))
        persist = top.enter_context(tc.tile_pool(name="persist", bufs=1))
        small = top.enter_context(tc.tile_pool(name="small", bufs=2))
        p1stack = ExitStack()
        p1sb = p1stack.enter_context(tc.tile_pool(name="p1sb", bufs=1))
        x4stack = ExitStack()
        x4pool = x4stack.enter_context(tc.tile_pool(name="x4pool", bufs=1))

        # ================= x2 chunk DMAs (sync queue, t=0) =============
        p2stack = ExitStack()
        p2s = p2stack.enter_context(tc.tile_pool(name="p2s", bufs=2))
        p2w = p2stack.enter_context(tc.tile_pool(name="p2w", bufs=1))
        HC = 14  # h rows per W-stage chunk
        x2st = []
        for hc in range(112 // HC):
            st = p2s.tile([128, HC * 112], BF16, tag="x2st", bufs=6,
                          name=f"x2st{hc}")
            nc.sync.dma_start(
                st[:], d_x2.ap()[:, hc * HC * 112:(hc + 1) * HC * 112]
            )
            x2st.append(st)

        # ---- packed weights: 2 DMAs on the scalar HWDGE queue ----
        wf = wpool.tile([128, 912], F32, tag="wf")
        nc.scalar.dma_start(wf[:], d_wf32.ap())
        wb = wpool.tile([128, 1008], BF16, tag="wb")
        nc.scalar.dma_start(wb[:], d_wbf.ap())
        f84B = (wb[:, 0:128], wb[:, 128:256], wb[0:64, 256:384])
        rh = wb[0:112, 384:608].rearrange("p (k o) -> p k o", k=4)
        rw = wb[0:112, 608:832].rearrange("p (k o) -> p k o", k=4)
        a112b = wb[:, 832:888]
        b112b = wb[:, 888:944]
        f83b = wb[:, 944:1008]
        qkAS = wf[:, 0:448]
        qkBS = wf[0:67, 448:896]
        fc8S = wf[:, 896:912]
        ident = wpool.tile([128, 128], F32, tag="ident")
        make_identity(nc, ident[:])

        # f32r weights: engine casts (DVE stream head; inputs land ~1.5us)
        qkAR = wpool.tile([128, 448], F32R, tag="qkAR")
        nc.vector.tensor_copy(qkAR[:], qkAS)
        qkBR = wpool.tile([67, 448], F32R, tag="qkBR")
        nc.vector.tensor_copy(qkBR[:], qkBS)
        fc8V = fc8S.rearrange("p (k o) -> p k o", k=4)  # [128, 4, 4] fp32

        # ---- early persistent activations ----
        camT5 = persist.tile([128, 125], BF16, tag="camT5")  # 25 blk x 5
        f_a = persist.tile([128, HW], F32R, tag="f_a")  # = f8_4
        f_b = persist.tile([67, HW], F32R, tag="f_b")  # = [f8_3(64); x_s(3)]
        x2r = persist.tile([128, HW], BF16, tag="x2r")
        x2rv = x2r[:].rearrange("p (h w) -> p h w", h=56)

        # h-block partition sizes: 24 x 128 + 1 x 64
        HBLK = [(i * 128, 128) for i in range(24)] + [(3072, 64)]
        # free-dim 512 chunks of 3136: 6 x 512 + 64
        NCH = [(i * 512, 512) for i in range(6)] + [(3072, 64)]

        # ================= x2 resize emit helpers =================
        x2w = p2w.tile([128, 112 * 56], BF16, tag="x2w")  # after W-resize
        x2wv = x2w[:].rearrange("p (h w) -> p h w", h=112)

        def emit_wchunk(hc):
            # whole chunks 1/3/5 on GpSimd (self-contained, off the DVE
            # critical path); everything else on DVE
            eng = nc.gpsimd if hc in (1, 3, 5) else nc.vector
            sv = x2st[hc][:].rearrange("p (h w) -> p h w", h=HC)
            dst = x2wv[:, hc * HC:(hc + 1) * HC, :]
            even = sv[:, :, 0:110:2]   # 55 taps
            odd = sv[:, :, 1:111:2]
            abc = a112b[:, 0:55].unsqueeze(1).broadcast_to([128, HC, 55])
            bbc = b112b[:, 0:55].unsqueeze(1).broadcast_to([128, HC, 55])
            t1 = p2s.tile([128, HC, 55], BF16, tag="t1", name=f"t1_{hc}")
            eng.tensor_tensor(t1[:], even, abc, op=ALU.mult)
            t2 = p2s.tile([128, HC, 55], BF16, tag="t2", name=f"t2_{hc}")
            eng.tensor_tensor(t2[:], odd, bbc, op=ALU.mult)
            eng.tensor_tensor(dst[:, :, 0:55], t1[:], t2[:], op=ALU.add)
            eng.tensor_copy(dst[:, :, 55:56], sv[:, :, 111:112])

        def emit_hquarter(q):
            j0, jl = 14 * q, (14 if q < 3 else 13)
            everow = x2wv[:, 2 * j0:2 * (j0 + jl) - 1:2, :]
            oddrow = x2wv[:, 2 * j0 + 1:2 * (j0 + jl):2, :]
            arow = a112b[:, j0:j0 + jl].unsqueeze(2).broadcast_to([128, jl, 56])
            brow = b112b[:, j0:j0 + jl].unsqueeze(2).broadcast_to([128, jl, 56])
            t3 = p2s.tile([128, 14, 56], BF16, tag="th1", name=f"th1_{q}")
            nc.vector.tensor_tensor(t3[:, 0:jl, :], everow, arow, op=ALU.mult)
            t4 = p2s.tile([128, 14, 56], BF16, tag="th2", name=f"th2_{q}")
            nc.vector.tensor_tensor(t4[:, 0:jl, :], oddrow, brow, op=ALU.mult)
            nc.vector.tensor_tensor(
                x2rv[:, j0:j0 + jl, :], t3[:, 0:jl, :], t4[:, 0:jl, :],
                op=ALU.add
            )
            if q == 3:
                nc.vector.tensor_copy(x2rv[:, 55:56, :], x2wv[:, 111:112, :])

        for hc in range(4):
            emit_wchunk(hc)

        # ======== f8_4 = relu(f84B.T @ deep3): slab-major, scalar q ========
        with tc.tile_pool(name="p5s", bufs=1) as p5s, \
             tc.tile_pool(name="p5p", bufs=1,
                          space=bass.MemorySpace.PSUM) as p5p:
            DCH = [(0, 128), (128, 128), (256, 64)]
            slabs = []
            for ci, (co, cl) in enumerate(DCH):
                sl = p5s.tile([cl, HW], BF16, tag=f"d3s{ci}", name=f"d3s{ci}")
                nc.scalar.dma_start(sl[:], d_deep3.ap()[co:co + cl, :])
                slabs.append(sl)
            fps = [
                p5p.tile([128, 512], F32, tag=f"f4psum{i}", name=f"f4psum{i}")
                for i in range(len(NCH))
            ]
            for ci, (co, cl) in enumerate(DCH):
                for (no, nl), fp in zip(NCH, fps):
                    nc.tensor.matmul(
                        fp[:, 0:nl], f84B[ci], slabs[ci][:, no:no + nl],
                        start=(ci == 0), stop=(ci == 2),
                    )
            for (no, nl), fp in zip(NCH, fps):
                nc.scalar.activation(f_a[:, no:no + nl], fp[:, 0:nl], AF.Relu)

        # ======== x4 slabs (fp32, queues split: s0 scalar, s1/s2 sync,
        # s3 scalar-after-xst) ========
        x4f = []
        for ck in range(4):
            sl = x4pool.tile([128, HW], F32, tag=f"x4f{ck}", name=f"x4f{ck}")
            x4f.append(sl)
        for ck, eng in ((0, nc.scalar), (1, nc.sync), (2, nc.sync)):
            eng.dma_start(x4f[ck][:], d_x4.ap()[128 * ck:128 * (ck + 1), :])

        # ================= P4: x -> x_s -> f_b[64:67] =================
        # copies routed to ScalarE so the DVE stream stays on the resize
        with tc.tile_pool(name="p4s", bufs=2) as p4s, \
             tc.tile_pool(name="p4sb", bufs=1) as p4sb, \
             tc.tile_pool(name="p4p", bufs=1, space=bass.MemorySpace.PSUM) as p4p:
            xh = p4sb.tile([56, 3, 448], BF16, tag="xh")
            xps = [
                p4p.tile([56, 448], F32, tag=f"xhp{c}", name=f"xhp{c}")
                for c in range(3)
            ]
            xdr = d_x.ap().rearrange("c h w -> h c w")
            for hc in range(4):
                st = p4s.tile([112, 3, 448], BF16, tag="xst")
                nc.scalar.dma_start(st[:], xdr[112 * hc:112 * (hc + 1), :, :])
                for c in range(3):
                    nc.tensor.matmul(
                        xps[c][:], rh[:, hc, :], st[:, c, :],
                        start=(hc == 0), stop=(hc == 3),
                    )
            for c in range(3):
                nc.scalar.copy(xh[:, c, :], xps[c][:])

            xhT = p4sb.tile([112, 12, 56], BF16, tag="xhT")
            idb = p4sb.tile([128, 128], BF16, tag="idb")
            nc.scalar.copy(idb[:], ident[:])
            for c in range(3):
                for wc in range(4):
                    tp = p4p.tile([112, 56], BF16, tag="xtp", bufs=2)
                    nc.tensor.transpose(
                        tp[:], xh[:, c, 112 * wc:112 * (wc + 1)], idb[0:56, 0:56]
                    )
                    nc.scalar.copy(xhT[:, c * 4 + wc, :], tp[:])
            xs3 = p4sb.tile([3, HW], F32, tag="xs3")
            for c in range(3):
                wp = p4p.tile([56, 56], F32, tag="xwp", bufs=2)
                for wc in range(4):
                    nc.tensor.matmul(
                        wp[:], xhT[:, c * 4 + wc, :], rw[:, wc, :],
                        start=(wc == 0), stop=(wc == 3),
                    )
                ws = p4s.tile([56, 56], F32, tag="xws")
                nc.scalar.copy(ws[:], wp[:])
                nc.sync.dma_start(xs3[c:c + 1, :], ws[:])
            nc.scalar.copy(f_b[64:67, :], xs3[:])

        # last x4 slab
        nc.scalar.dma_start(x4f[3][:], d_x4.ap()[384:512, :])

        # late persistent activations (emitted here so their SBUF doesn't
        # collide with the staging/resize pools' peak)
        qA = persist.tile([128, HW], BF16, tag="qA")
        qBp = persist.tile([128, HW], BF16, tag="qBp")  # rows 64:128 zero
        kA = persist.tile([128, HW], BF16, tag="kA")
        kBp = persist.tile([128, HW], BF16, tag="kBp")  # rows 64:128 zero
        nc.gpsimd.memset(qBp[64:128, :], 0.0)
        nc.gpsimd.memset(kBp[64:128, :], 0.0)

        # ---- rest of the resize: W chunks 4-7 + H quarters pipelined ----
        emit_hquarter(0)
        emit_wchunk(4)
        emit_wchunk(5)
        emit_hquarter(1)
        emit_wchunk(6)
        emit_wchunk(7)
        emit_hquarter(2)
        emit_hquarter(3)
        p2stack.close()

        # ======== cam = fc8.T @ _4 (exact fp32, slab-major) ========
        # Slab-major so the PE chews each slab as its DMA lands; fp32
        # moving operand runs at 1/4 rate but exactness is mandatory.
        cam = p1sb.tile([4, HW], F32, tag="cam")
        with tc.tile_pool(name="p1p", bufs=1,
                          space=bass.MemorySpace.PSUM) as p1p:
            cps = [
                p1p.tile([4, 512], F32, tag=f"campsum{i}", name=f"campsum{i}")
                for i in range(len(NCH))
            ]
            for ck in range(4):
                for (no, nl), cp in zip(NCH, cps):
                    nc.tensor.matmul(
                        cp[:, 0:nl], fc8V[:, ck, :], x4f[ck][:, no:no + nl],
                        start=(ck == 0), stop=(ck == 3),
                    )
            for (no, nl), cp in zip(NCH, cps):
                nc.scalar.copy(cam[:, no:no + nl], cp[:, 0:nl])
        x4stack.close()

        # ==== f8_3 (bf16) then q/k (per-dst batched A/B passes) ====
        # All A stationaries are M=128 slices of the 448-wide padded pack,
        # so the only weight-shape transitions are A<->B per dst.
        MCH = [(0, 128), (128, 64), (192, 128), (320, 64)]  # qA qB kA kB
        DSTS = (qA, qBp, kA, kBp)
        with tc.tile_pool(name="p3p", bufs=2,
                          space=bass.MemorySpace.PSUM) as p3p:
            for ci, (no, nl) in enumerate(NCH):
                fp3 = p3p.tile([64, 512], F32, tag="f3psum")
                nc.tensor.matmul(
                    fp3[:, 0:nl], f83b[:], x2r[:, no:no + nl],
                    start=True, stop=True,
                )
                nc.scalar.activation(
                    f_b[0:64, no:no + nl], fp3[:, 0:nl], AF.Relu
                )
        with tc.tile_pool(name="p6p", bufs=1,
                          space=bass.MemorySpace.PSUM) as p6p:
            for di, (mo, ml) in enumerate(MCH):
                qps = []
                for ci, (no, nl) in enumerate(NCH):
                    qp = p6p.tile([128, 512], F32, tag=f"qkp{ci}",
                                  name=f"qkp{di}_{ci}")
                    qps.append(qp)
                    nc.tensor.matmul(
                        qp[:, 0:nl], qkAR[:, mo:mo + 128],
                        f_a[:, no:no + nl],
                        start=True, stop=False,
                    )
                for (no, nl), qp in zip(NCH, qps):
                    nc.tensor.matmul(
                        qp[:, 0:nl], qkBR[:, mo:mo + 128],
                        f_b[:, no:no + nl],
                        start=False, stop=True,
                    )
                for ci, ((no, nl), qp) in enumerate(zip(NCH, qps)):
                    if ci % 2 == 0:
                        nc.vector.tensor_copy(
                            DSTS[di][0:ml, no:no + nl], qp[0:ml, 0:nl]
                        )
                    else:
                        nc.scalar.copy(
                            DSTS[di][0:ml, no:no + nl], qp[0:ml, 0:nl]
                        )

        mn = small.tile([4, 1], F32, tag="mn")
        mx = small.tile([4, 1], F32, tag="mx")
        nc.vector.tensor_reduce(
            mn[:], cam[:], axis=mybir.AxisListType.X, op=ALU.min
        )
        nc.vector.tensor_reduce(
            mx[:], cam[:], axis=mybir.AxisListType.X, op=ALU.max
        )
        rng = small.tile([4, 1], F32, tag="rng")
        nc.vector.tensor_tensor(rng[:], mx[:], mn[:], op=ALU.subtract)
        nc.vector.tensor_scalar_add(rng[:], rng[:], EPS)
        rs = small.tile([4, 1], F32, tag="rs")
        nc.vector.reciprocal(rs[:], rng[:])
        nbias = small.tile([4, 1], F32, tag="nbias")
        nc.vector.tensor_tensor(nbias[:], mn[:], rs[:], op=ALU.mult)
        nc.vector.tensor_scalar(nbias[:], nbias[:], -1.0, 0.0,
                                op0=ALU.mult, op1=ALU.add)
        norm = p1sb.tile([4, HW], F32, tag="norm")
        # norm = cam*rs - mn*rs in one ScalarE op (free affine)
        nc.scalar.activation(norm[:], cam[:], AF.Identity,
                             bias=nbias[:], scale=rs[:])

        camTall = p1sb.tile([128, 25, 4], F32, tag="camTall")
        nc.vector.memset(camTall[64:128, 24, :], 0.0)
        with tc.tile_pool(name="p1t", bufs=2,
                          space=bass.MemorySpace.PSUM) as p1t:
            for bi, (ho, hl) in enumerate(HBLK):
                tp = p1t.tile([128, 4], F32, tag="tpsum")
                nc.tensor.transpose(
                    tp[0:hl, :], norm[:, ho:ho + hl], ident[0:4, 0:4]
                )
                nc.vector.tensor_copy(camTall[0:hl, bi, :], tp[0:hl, :])
        c5v = camT5[:].rearrange("p (b f) -> p b f", f=5)
        nc.vector.memset(c5v[:, :, 4], 1.0)
        fm = p1sb.tile([128, 25], F32, tag="fm")
        nc.vector.tensor_reduce(
            fm[:], camTall[:, :, 1:4],
            axis=mybir.AxisListType.X, op=ALU.max
        )
        nc.vector.tensor_scalar(
            c5v[:, :, 0], fm[:], -1.0, 1.0,
            op0=ALU.mult, op1=ALU.add
        )
        msk = p1sb.tile([128, 25, 3], F32, tag="msk")
        fmb = fm[:].unsqueeze(2).broadcast_to([128, 25, 3])
        nc.vector.tensor_tensor(
            msk[:], camTall[:, :, 1:4], fmb, op=ALU.is_ge
        )
        nc.vector.tensor_tensor(
            c5v[:, :, 1:4], camTall[:, :, 1:4], msk[:], op=ALU.mult
        )

        # ================= P7: attention =================
        with tc.tile_pool(name="p7e", bufs=20) as p7e, \
             tc.tile_pool(name="p7r", bufs=1) as p7r, \
             tc.tile_pool(name="p7s", bufs=3, space=bass.MemorySpace.PSUM) as p7s, \
             tc.tile_pool(name="p7o", bufs=1, space=bass.MemorySpace.PSUM) as p7o:
            KSUP = [(784 * i, 784) for i in range(4)]
            KCH = [(0, 512), (512, 272)]

            def emit_2nd_batch(st, blocks):
                if st["pout"] is None:
                    st["pout"] = p7o.tile([5, 784], F32, tag="pout",
                                          name="pout")
                for pb in blocks:
                    pho, phl = HBLK[pb]
                    for kb, kbl in KCH:
                        nc.tensor.matmul(
                            st["pout"][:, kb:kb + kbl],
                            camT5[0:phl, pb * 5:pb * 5 + 5],
                            st["ets"][pb][0:phl, kb:kb + kbl],
                            start=(pb == 0), stop=(pb == 24),
                        )

            def emit_epilogue(st):
                # broadcast the fused denominator (row 4) to partitions
                # 0-3 via 4 sync-queue SBUF DMAs, reciprocal, multiply
                ko, kl, pout = st["ko"], st["kl"], st["pout"]
                ot5 = p7r.tile([5, 784], F32, tag="ot5")
                nc.vector.tensor_copy(ot5[:], pout[:])
                den4 = p7r.tile([4, 784], F32, tag="den4")
                for r in range(4):
                    nc.sync.dma_start(den4[r:r + 1, :], ot5[4:5, :])
                rcp4 = p7r.tile([4, 784], F32, tag="rcp4")
                rsc4 = p7r.tile([4, 784], F32, tag="rsc4")
                nc.vector.reciprocal_approx_accurate(
                    rcp4[:], den4[:], rsc4[:]
                )
                ob4 = p7r.tile([4, 784], F32, tag="ob4")
                nc.vector.tensor_tensor(
                    ob4[:], ot5[0:4, :], rcp4[:], op=ALU.mult
                )
                nc.sync.dma_start(d_out.ap()[:, ko:ko + kl], ob4[:])

            prev = None  # previous superblock's state
            for si, (ko, kl) in enumerate(KSUP):
                st = {"ko": ko, "kl": kl, "ets": [], "pout": None}
                for bi, (ho, hl) in enumerate(HBLK):
                    sp = p7s.tile([128, 784], F32, tag="spsum")
                    for kb, kbl in KCH:
                        nc.tensor.matmul(
                            sp[0:hl, kb:kb + kbl], qA[:, ho:ho + hl],
                            kA[:, ko + kb:ko + kb + kbl],
                            start=True, stop=False,
                        )
                    for kb, kbl in KCH:
                        nc.tensor.matmul(
                            sp[0:hl, kb:kb + kbl], qBp[:, ho:ho + hl],
                            kBp[:, ko + kb:ko + kb + kbl],
                            start=False, stop=True,
                        )
                    et = p7e.tile([128, 784], BF16, tag="exptile")
                    nc.scalar.activation(et[0:hl, :], sp[0:hl, :], AF.Exp)
                    st["ets"].append(et)
                    if bi == 4 and prev is not None:
                        emit_2nd_batch(prev, range(20, 25))
                        emit_epilogue(prev)
                        prev = None
                    if bi in (9, 14, 19, 24):
                        emit_2nd_batch(st, range(bi - 9, bi - 4))
                prev = st
            emit_2nd_batch(prev, range(20, 25))
            emit_epilogue(prev)
        p1stack.close()

    nc.compile()
    return nc


def _get_program():
    if "nc" not in _CACHE:
        _CACHE["nc"] = _build_program()
    return _CACHE["nc"]


def _host_prep(inputs: dict) -> list[dict]:
    import ml_dtypes

    BFNP = ml_dtypes.bfloat16
    x = np.ascontiguousarray(np.asarray(inputs["x"], np.float32))
    x2 = np.ascontiguousarray(np.asarray(inputs["x2"], np.float32))
    deep3 = np.ascontiguousarray(np.asarray(inputs["deep3"], np.float32))
    _4 = np.ascontiguousarray(np.asarray(inputs["_4"], np.float32))
    fc8_w = np.asarray(inputs["fc8_w"], np.float32)
    f83_w = np.asarray(inputs["f83_w"], np.float32)
    f84_w = np.asarray(inputs["f84_w"], np.float32)
    f91_w = np.asarray(inputs["f91_w"], np.float32)
    f92_w = np.asarray(inputs["f92_w"], np.float32)

    n = x.shape[0]
    # f channel permutation: [f8_4 (128), f8_3 (64), x_s (3)]
    perm = np.concatenate([np.arange(67, 195), np.arange(3, 67), np.arange(3)])
    wqk = np.concatenate([f91_w, f92_w], axis=0)[:, perm]  # [384, 195]
    wqkT = np.ascontiguousarray(wqk.T)  # [195, 384]
    a112, b112 = _resize_coeffs_112()

    # fp32 pack [128, 912]: qkA-pad448 | qkB-pad448 | fc8-pack16
    wf32 = np.zeros((128, 912), np.float32)
    wf32[:, 0:384] = wqkT[0:128]
    wf32[0:67, 448:832] = wqkT[128:195]
    wf32[:, 896:912] = fc8_w.T.reshape(4, 128, 4).transpose(1, 0, 2).reshape(128, 16)

    # bf16 pack [128, 1008]: f84(3x128) | rh(224) | rw(224) | a112 | b112 | f83
    rh448 = _resize_mat(448, 56).astype(np.float32)
    rhp = rh448.reshape(4, 112, 56).transpose(1, 0, 2).reshape(112, 224)
    wbf = np.zeros((128, 1008), np.float32)
    f84T = f84_w.T  # [320, 128]
    wbf[:, 0:128] = f84T[0:128]
    wbf[:, 128:256] = f84T[128:256]
    wbf[0:64, 256:384] = f84T[256:320]
    wbf[0:112, 384:608] = rhp
    wbf[0:112, 608:832] = rhp
    wbf[:, 832:888] = np.broadcast_to(a112, (128, 56))
    wbf[:, 888:944] = np.broadcast_to(b112, (128, 56))
    wbf[:, 944:1008] = f83_w.T  # [128, 64]
    wbf = wbf.astype(BFNP)

    x = x.astype(BFNP)
    x2 = x2.reshape(n, 128, 112 * 112).astype(BFNP)
    deep3 = deep3.reshape(n, 320, HW).astype(BFNP)
    _4 = _4.reshape(n, 512, HW)

    shared = {"wf32": wf32, "wbf": wbf}
    in_maps = []
    for i in range(n):
        m = dict(shared)
        m["x4"] = _4[i]
        m["deep3"] = deep3[i]
        m["x2"] = x2[i]
        m["x"] = x[i]
        in_maps.append(m)
    return in_maps


def _install_ntff_hook() -> bool:
    """Register the NTFF profile hook that the agent image's antenv lacks."""
    try:
        import types

        import antenv

        if "antenv.axon_hooks" not in sys.modules:
            mod = types.ModuleType("antenv.axon_hooks")
            store = {"h": None}
            mod.set_axon_ntff_profile_hook = lambda h: store.update(h=h)
            mod.get_axon_ntff_profile_hook = lambda: store["h"]
            sys.modules["antenv.axon_hooks"] = mod
            antenv.axon_hooks = mod
            from trn_agent_boot.trn_boot import _ntff_profile_via_ctypes

            hook = _ntff_profile_via_ctypes("/opt/axon/libaxon_pjrt.so")
            if hook is None:
                return False
            mod.set_axon_ntff_profile_hook(hook)
        return sys.modules["antenv.axon_hooks"].get_axon_ntff_profile_hook() is not None
    except Exception as e:  # profiling is best-effort
        print(f"ntff hook install failed: {e}", file=sys.stderr)
        return False


def kernel(**inputs) -> np.ndarray:
    nc = _get_program()
    in_maps = _host_prep(inputs)
    trace = bool(int(os.environ.get("KERNEL_PROFILE", "0")))
    if trace:
        trace = _install_ntff_hook()
    res = run_bass_kernel_spmd(nc, in_maps, core_ids=list(range(N_CORES)),
                               trace=trace)
    _CACHE["last_result"] = res
    out = np.stack([r["out"] for r in res.results]).reshape(8, 4, 56, 56)
    return out.astype(np.float32)
